# revision 30
# baseline (speedup 1.0000x reference)
"""Kendall distance kernel for Trainium2, SPMD over 8 NeuronCores.

Math: for X (B=64, T=256, N=64),
  C[i,j] = sum_{a,b,t} sign(X[b,t,i]-X[a,t,i]) * sign(X[b,t,j]-X[a,t,j])
         = 2 * sum_{a<b,t} (...)          (diagonal a=b contributes 0)
  D = (1 - C/2016) * (1 - eye(N));  output = broadcast D to (B, N, N).

Device work: the Gram matrix of the +-1 sign tensor over all unordered
batch pairs (2016 pairs x 256 t = 516096 rows), sharded across 8 cores
by cyclic batch-difference class: core c handles ring-offset classes
d in {4c+1 .. 4c+4}.  Classes 1..31 cover each unordered pair exactly
once; class 32 (core 7, slot 3) covers its 32 pairs twice, so slot-3
chunks accumulate into a second PSUM accumulator which the host halves
for core 7.

SPMD-uniform program: every core runs identical code.  The per-core
class offset is encoded in the DATA: inputs are R (64 blocks, natural
order) and 4 overlapping 19-block windows of roll(R, -(4c+1)) extended,
where R is the per-(t,i)-column RANK transform of X (exact in bf16;
sign(rank diff) == sign(value diff) except exact value ties, which the
host corrects - see _tie_correction).

Per chunk (16 blocks x one class): DVE bf16 subtract (2x mode), then
either ACT Sign or a DVE min/max clamp to +-1 (4x mode) - chunks are
split between the two engines to balance their load.  PE consumes
sign tiles PAIRED: W = [S_2k | S_2k+1] (128x128 bf16) in a single
FWL-eligible matmul; the diagonal 64x64 blocks of W^T W are the two
tiles' Grams (off-diagonal blocks are discarded by the host).
"""

import numpy as np
import ml_dtypes

import concourse.bass as bass  # noqa: F401
import concourse.bacc as bacc
import concourse.tile as tile
from concourse import mybir
from concourse.bass_utils import run_bass_kernel_spmd

B, T, N = 64, 256, 64
P = 128
TH = T // P                   # 2
NCORES = 8
NSLOT = 4
SUB = 4                       # chunks per slot
CB = B // SUB                 # blocks per chunk (16)
WB = CB + NSLOT - 1           # blocks per x2 window (19)
BFD = TH * N                  # free elems per block (128)
CFD = CB * BFD                # free elems per chunk (2048)
PMT = CB * TH // 2            # paired matmuls per chunk (16)
PAIRS_HALF = 1008.0

_CACHE = {}


def _build_nc():
    nc = bacc.Bacc(
        "TRN2",
        target_bir_lowering=False,
        debug=False,
        num_devices=NCORES,
    )
    f32 = mybir.dt.float32
    bf16 = mybir.dt.bfloat16
    x1_dram = [
        nc.dram_tensor(f"x1s{s}", [P, CFD], bf16, kind="ExternalInput")
        for s in range(SUB)
    ]
    x2_dram = [
        nc.dram_tensor(f"x2s{s}", [P, WB * BFD], bf16, kind="ExternalInput")
        for s in range(SUB)
    ]
    out_dram = nc.dram_tensor("out", [P, 2 * P], f32, kind="ExternalOutput")

    with tile.TileContext(nc) as tc:
        with (
            tc.tile_pool(name="xpool", bufs=1) as xpool,
            tc.tile_pool(name="dpool", bufs=6) as dpool,
            tc.tile_pool(name="spool", bufs=6) as spool,
            tc.tile_pool(name="psum", bufs=2, space="PSUM") as psum,
            tc.tile_pool(name="opool", bufs=1) as opool,
        ):
            x1t = [
                xpool.tile([P, CFD], bf16, tag=f"x1t{s}", name=f"x1t{s}")
                for s in range(SUB)
            ]
            x2t = [
                xpool.tile([P, WB * BFD], bf16, tag=f"x2t{s}", name=f"x2t{s}")
                for s in range(SUB)
            ]
            # contiguous per-tile transfers, spread across engine queues so
            # the pieces stream in parallel and complete in consumption order
            # gpsimd + scalar DMA queues each sustain ~150GB/s concurrently
            # (sync's is much slower) - alternate so each chunk's two pieces
            # land at matching FIFO positions and arrive together, in order.
            for s in range(SUB):
                a, b = (nc.gpsimd, nc.scalar) if s % 2 == 0 else (nc.scalar, nc.gpsimd)
                if s == 0:
                    # split the first tile pair so the first half-chunk's
                    # data lands ~2us sooner
                    h2 = (CB // 2 + NSLOT - 1) * BFD
                    a.dma_start(x2t[s][:, 0:h2], x2_dram[s][:, 0:h2])
                    a.dma_start(x2t[s][:, h2:WB * BFD], x2_dram[s][:, h2:WB * BFD])
                    h1 = (CB // 2) * BFD
                    b.dma_start(x1t[s][:, 0:h1], x1_dram[s][:, 0:h1])
                    b.dma_start(x1t[s][:, h1:CFD], x1_dram[s][:, h1:CFD])
                else:
                    a.dma_start(x2t[s][:, :], x2_dram[s][:, :])
                    b.dma_start(x1t[s][:, :], x1_dram[s][:, :])

            c1_ps = psum.tile([P, P], f32, tag="c1")
            c2_ps = psum.tile([P, P], f32, tag="c2")
            n1 = (NSLOT - 1) * SUB * PMT
            n2 = SUB * PMT
            # first and last chunks run as halves: shorter pipeline ramp-up
            # (first data arrives sooner) and ramp-down (shorter serial
            # subtract->sign->matmul tail)
            chunks = []
            for s in range(SUB):
                for j in range(NSLOT):
                    if (s == 0 and j == 0) or (s == SUB - 1 and j == NSLOT - 1):
                        chunks.append((s, j, 0, CB // 2))
                        chunks.append((s, j, CB // 2, CB - CB // 2))
                    else:
                        chunks.append((s, j, 0, CB))
            k1 = k2 = 0
            for idx, (s, j, lo, nb) in enumerate(chunks):
                fd = nb * BFD
                diff = dpool.tile([P, fd], bf16, tag="diff", name=f"diff{idx}")
                sign = spool.tile([P, fd], bf16, tag="sign", name=f"sign{idx}")
                nc.vector.tensor_tensor(
                    diff[:, :],
                    x2t[s][:, (j + lo) * BFD:(j + lo) * BFD + fd],
                    x1t[s][:, lo * BFD:lo * BFD + fd],
                    op=mybir.AluOpType.subtract,
                )
                if idx % 3 == 1:
                    # DVE route: clamp nonzero integer diffs to +-1
                    nc.vector.tensor_scalar(
                        sign[:, :],
                        diff[:, :],
                        1.0,
                        -1.0,
                        op0=mybir.AluOpType.min,
                        op1=mybir.AluOpType.max,
                    )
                else:
                    nc.scalar.activation(
                        sign[:, :],
                        diff[:, :],
                        mybir.ActivationFunctionType.Sign,
                    )
                for m in range(nb * TH // 2):
                    w_tile = sign[:, m * P:(m + 1) * P]
                    if j < NSLOT - 1:
                        st, sp = k1 == 0, k1 == n1 - 1
                        k1 += 1
                        acc = c1_ps
                    else:
                        st, sp = k2 == 0, k2 == n2 - 1
                        k2 += 1
                        acc = c2_ps
                    nc.tensor.matmul(
                        acc[:, :], w_tile, w_tile, start=st, stop=sp
                    )

            out_sb = opool.tile([P, 2 * P], f32)
            nc.vector.tensor_copy(out_sb[:, 0:P], c1_ps[:, :])
            nc.scalar.activation(
                out_sb[:, P:2 * P],
                c2_ps[:, :],
                mybir.ActivationFunctionType.Copy,
            )
            nc.sync.dma_start(out_dram[:, :], out_sb[:, :])

    nc.compile()
    return nc


def _build_nc_raw():
    """Raw bacc pipeline (no TileContext): manual semaphores, so the ~10us
    Tile epilogue (drain + 2 all-engine barriers + sem clears) disappears.

    Engine plan per core (16 chunks, idx = s*4+j):
      gpsimd : 4 input DMAs (its HW queue, ~160GB/s)
      scalar : 4 input DMAs + 11 ACT Sign chunks + final c2 copy
      vector : 16 subtracts + 5 clamp chunks + final c1 copy
      tensor : 256 paired Gram matmuls
      sync   : output DMA
    """
    nc = bacc.Bacc(
        "TRN2",
        target_bir_lowering=False,
        debug=False,
        num_devices=NCORES,
    )
    f32 = mybir.dt.float32
    bf16 = mybir.dt.bfloat16
    x1_dram = [
        nc.dram_tensor(f"x1s{s}", [P, CFD], bf16, kind="ExternalInput")
        for s in range(SUB)
    ]
    x2_dram = [
        nc.dram_tensor(f"x2s{s}", [P, WB * BFD], bf16, kind="ExternalInput")
        for s in range(SUB)
    ]
    out_dram = nc.dram_tensor("out", [P, 2 * P], f32, kind="ExternalOutput")

    NB = 4                       # diff/sign ring depth
    NCHUNK = SUB * NSLOT
    is_dve = [idx % 3 == 1 for idx in range(NCHUNK)]
    # running counts of ACT-/DVE-route chunks with index <= idx
    a_cnt, d_cnt, a, d = [], [], 0, 0
    for idx in range(NCHUNK):
        if is_dve[idx]:
            d += 1
        else:
            a += 1
        a_cnt.append(a)
        d_cnt.append(d)

    x1t = [nc.alloc_sbuf_tensor(f"x1t{s}", [P, CFD], bf16) for s in range(SUB)]
    x2t = [nc.alloc_sbuf_tensor(f"x2t{s}", [P, WB * BFD], bf16) for s in range(SUB)]
    diff = [nc.alloc_sbuf_tensor(f"diff{k}", [P, CFD], bf16) for k in range(NB)]
    sign = [nc.alloc_sbuf_tensor(f"sign{k}", [P, CFD], bf16) for k in range(NB)]
    out_sb = nc.alloc_sbuf_tensor("out_sb", [P, 2 * P], f32)
    c1_ps = nc.alloc_psum_tensor("c1_ps", [P, P], f32)
    c2_ps = nc.alloc_psum_tensor("c2_ps", [P, P], f32)

    n1 = (NSLOT - 1) * SUB * PMT
    n2 = SUB * PMT

    from contextlib import ExitStack

    with ExitStack() as ctx:
        gd = [ctx.enter_context(nc.semaphore(f"gd{s}")) for s in range(SUB)]
        sd = [ctx.enter_context(nc.semaphore(f"sd{s}")) for s in range(SUB)]
        diff_sem = ctx.enter_context(nc.semaphore("diff_sem"))
        act_sem = ctx.enter_context(nc.semaphore("act_sem"))
        dvs_sem = ctx.enter_context(nc.semaphore("dvs_sem"))
        mm_sem = ctx.enter_context(nc.semaphore("mm_sem"))
        cp_sem = ctx.enter_context(nc.semaphore("cp_sem"))
        odma_sem = ctx.enter_context(nc.semaphore("odma_sem"))
        block = ctx.enter_context(nc.Block())

        @block.gpsimd
        def _(gpsimd):
            for s in range(SUB):
                src = x2_dram[s] if s % 2 == 0 else x1_dram[s]
                dst = x2t[s] if s % 2 == 0 else x1t[s]
                gpsimd.dma_start(dst.ap(), src.ap()).then_inc(gd[s], 16)

        @block.scalar
        def _(scalar):
            for s in range(SUB):
                src = x1_dram[s] if s % 2 == 0 else x2_dram[s]
                dst = x1t[s] if s % 2 == 0 else x2t[s]
                scalar.dma_start(dst.ap(), src.ap()).then_inc(sd[s], 16)
            for idx in range(NCHUNK):
                if is_dve[idx]:
                    continue
                scalar.wait_ge(diff_sem, idx + 1)
                if idx >= NB:
                    scalar.wait_ge(mm_sem, idx - NB + 1)
                scalar.activation(
                    sign[idx % NB].ap(),
                    diff[idx % NB].ap(),
                    mybir.ActivationFunctionType.Sign,
                ).then_inc(act_sem, 1)
            scalar.wait_ge(mm_sem, NCHUNK)
            scalar.activation(
                out_sb.ap()[:, P:2 * P],
                c2_ps.ap(),
                mybir.ActivationFunctionType.Copy,
            ).then_inc(cp_sem, 1)

        @block.vector
        def _(vector):
            for idx in range(NCHUNK):
                s, j = divmod(idx, NSLOT)
                if j == 0:
                    vector.wait_ge(gd[s], 16)
                    vector.wait_ge(sd[s], 16)
                prev = idx - NB
                if prev >= 0 and not is_dve[prev]:
                    vector.wait_ge(act_sem, a_cnt[prev])
                vector.tensor_tensor(
                    diff[idx % NB].ap(),
                    x2t[s].ap()[:, j * BFD:j * BFD + CFD],
                    x1t[s].ap(),
                    op=mybir.AluOpType.subtract,
                ).then_inc(diff_sem, 1)
                if is_dve[idx]:
                    vector.wait_ge(diff_sem, idx + 1)
                    if idx >= NB:
                        vector.wait_ge(mm_sem, idx - NB + 1)
                    vector.tensor_scalar(
                        sign[idx % NB].ap(),
                        diff[idx % NB].ap(),
                        1.0,
                        -1.0,
                        op0=mybir.AluOpType.min,
                        op1=mybir.AluOpType.max,
                    ).then_inc(dvs_sem, 1)
            vector.wait_ge(mm_sem, NCHUNK)
            vector.tensor_copy(out_sb.ap()[:, 0:P], c1_ps.ap()).then_inc(cp_sem, 1)

        @block.tensor
        def _(tensor):
            k1 = k2 = 0
            for idx in range(NCHUNK):
                j = idx % NSLOT
                tensor.wait_ge(act_sem, a_cnt[idx])
                tensor.wait_ge(dvs_sem, d_cnt[idx])
                for m in range(PMT):
                    w_tile = sign[idx % NB].ap()[:, m * P:(m + 1) * P]
                    if j < NSLOT - 1:
                        st, sp = k1 == 0, k1 == n1 - 1
                        k1 += 1
                        acc = c1_ps
                    else:
                        st, sp = k2 == 0, k2 == n2 - 1
                        k2 += 1
                        acc = c2_ps
                    mm = tensor.matmul(
                        acc.ap(), w_tile, w_tile, start=st, stop=sp
                    )
                    if m == PMT - 1:
                        mm.then_inc(mm_sem, 1)

        @block.sync
        def _(sync):
            sync.wait_ge(cp_sem, 2)
            sync.dma_start(out_dram.ap(), out_sb.ap()).then_inc(odma_sem, 16)
            sync.wait_ge(odma_sem, 16)

    nc.compile()
    return nc


def _get_nc():
    # The TileContext build measures ~43us vs ~46us for the manual-semaphore
    # build (_build_nc_raw): the end-of-kernel barrier/sem traffic is
    # Bass/NRT-level (present in both), and Tile's finer-grained deps
    # overlap the chunk pipeline better.
    if "nc" not in _CACHE:
        _CACHE["nc"] = _build_nc()
    return _CACHE["nc"]


def _ranks(X):
    """Per-(t,i)-column batch ranks, 0..B-1, exact in bf16."""
    order = np.argsort(X, axis=0, kind="stable")
    ranks = np.empty_like(order)
    np.put_along_axis(
        ranks, order, np.arange(B, dtype=order.dtype)[:, None, None], axis=0
    )
    return ranks.astype(np.float32)


def _to_sbuf_layout(blocks):
    nb = blocks.shape[0]
    return np.ascontiguousarray(
        blocks.reshape(nb, TH, P, N)
        .transpose(2, 0, 1, 3)
        .reshape(P, nb * BFD)
        .astype(ml_dtypes.bfloat16)
    )


def _prep_core_inputs(R, c):
    r = np.roll(R, -(NSLOT * c + 1), axis=0)
    ext = np.concatenate([r, r[: NSLOT - 1]], axis=0)  # 67 blocks
    ins = {}
    for s in range(SUB):
        ins[f"x1s{s}"] = _to_sbuf_layout(R[CB * s:CB * (s + 1)])
        ins[f"x2s{s}"] = _to_sbuf_layout(ext[CB * s:CB * s + WB])
    return ins


def _tie_correction(X, ranks):
    """Exact fix for within-column value ties: the rank-sign kernel counts
    sign(rank diff)=+-1 where the true sign is 0."""
    C_fix = np.zeros((N, N), dtype=np.float64)
    Xs = np.sort(X, axis=0)
    t_idx, i_idx = np.nonzero((Xs[1:] == Xs[:-1]).any(axis=0))
    events = {}
    for t, i in zip(t_idx, i_idx):
        col = X[:, t, i]
        order = np.argsort(col, kind="stable")
        sc = col[order]
        for k in np.nonzero(sc[1:] == sc[:-1])[0]:
            a, b = order[k], order[k + 1]
            events.setdefault((min(a, b), max(a, b), t), []).append(i)
    for (a, b, t), cols in events.items():
        shat = np.sign(ranks[b, t, :] - ranks[a, t, :])
        W = np.outer(shat, shat)
        mask = np.zeros((N, N), dtype=bool)
        mask[cols, :] = True
        mask[:, cols] = True
        C_fix += W * mask
    return C_fix.astype(np.float32)


def kernel(**inputs) -> np.ndarray:
    X = np.asarray(inputs["inputs"], dtype=np.float32)
    R = _ranks(X)
    nc = _get_nc()
    in_maps = [_prep_core_inputs(R, c) for c in range(NCORES)]
    res = run_bass_kernel_spmd(nc, in_maps, core_ids=list(range(NCORES)))
    C_half = np.zeros((N, N), dtype=np.float32)
    for c, r in enumerate(res.results):
        o = r["out"]
        C_half += o[0:N, 0:N] + o[N:P, N:P]
        w = np.float32(0.5) if c == NCORES - 1 else np.float32(1.0)
        C_half += (o[0:N, P:P + N] + o[N:P, P + N:2 * P]) * w
    C_half -= _tie_correction(X, R)
    D = (1.0 - C_half / np.float32(PAIRS_HALF)) * (
        1.0 - np.eye(N, dtype=np.float32)
    )
    return np.ascontiguousarray(
        np.broadcast_to(D[None].astype(np.float32), (B, N, N))
    )


# revision 31
# speedup vs baseline: 1.0206x; 1.0206x over previous
"""Kendall distance kernel for Trainium2, SPMD over 8 NeuronCores.

Math: for X (B=64, T=256, N=64),
  C[i,j] = sum_{a,b,t} sign(X[b,t,i]-X[a,t,i]) * sign(X[b,t,j]-X[a,t,j])
         = 2 * sum_{a<b,t} (...)          (diagonal a=b contributes 0)
  D = (1 - C/2016) * (1 - eye(N));  output = broadcast D to (B, N, N).

Device work: the Gram matrix of the +-1 sign tensor over all unordered
batch pairs (2016 pairs x 256 t = 516096 rows), sharded across 8 cores
by cyclic batch-difference class: core c handles ring-offset classes
d in {4c+1 .. 4c+4}.  Classes 1..31 cover each unordered pair exactly
once; class 32 (core 7, slot 3) covers its 32 pairs twice, so slot-3
chunks accumulate into a second PSUM accumulator which the host halves
for core 7.

SPMD-uniform program: every core runs identical code.  The per-core
class offset is encoded in the DATA: inputs are R (64 blocks, natural
order) and 4 overlapping 19-block windows of roll(R, -(4c+1)) extended,
where R is the per-(t,i)-column RANK transform of X (exact in bf16;
sign(rank diff) == sign(value diff) except exact value ties, which the
host corrects - see _tie_correction).

Per chunk (16 blocks x one class): DVE bf16 subtract (2x mode), then
either ACT Sign or a DVE min/max clamp to +-1 (4x mode) - chunks are
split between the two engines to balance their load.  PE consumes
sign tiles PAIRED: W = [S_2k | S_2k+1] (128x128 bf16) in a single
FWL-eligible matmul; the diagonal 64x64 blocks of W^T W are the two
tiles' Grams (off-diagonal blocks are discarded by the host).
"""

import numpy as np
import ml_dtypes

import concourse.bass as bass  # noqa: F401
import concourse.bacc as bacc
import concourse.tile as tile
from concourse import mybir
from concourse.bass_utils import run_bass_kernel_spmd

B, T, N = 64, 256, 64
P = 128
TH = T // P                   # 2
NCORES = 8
NSLOT = 4
SUB = 4                       # chunks per slot
CB = B // SUB                 # blocks per chunk (16)
WB = CB + NSLOT - 1           # blocks per x2 window (19)
BFD = TH * N                  # free elems per block (128)
CFD = CB * BFD                # free elems per chunk (2048)
PMT = CB * TH // 2            # paired matmuls per chunk (16)
PAIRS_HALF = 1008.0

_CACHE = {}


def _build_nc():
    nc = bacc.Bacc(
        "TRN2",
        target_bir_lowering=False,
        debug=False,
        num_devices=NCORES,
    )
    f32 = mybir.dt.float32
    bf16 = mybir.dt.bfloat16
    x1_dram = [
        nc.dram_tensor(f"x1s{s}", [P, CFD], bf16, kind="ExternalInput")
        for s in range(SUB)
    ]
    x2_dram = [
        nc.dram_tensor(f"x2s{s}", [P, WB * BFD], bf16, kind="ExternalInput")
        for s in range(SUB)
    ]
    out_dram = nc.dram_tensor("out", [P, 2 * P], f32, kind="ExternalOutput")

    with tile.TileContext(nc) as tc:
        with (
            tc.tile_pool(name="xpool", bufs=1) as xpool,
            tc.tile_pool(name="dpool", bufs=6) as dpool,
            tc.tile_pool(name="spool", bufs=6) as spool,
            tc.tile_pool(name="psum", bufs=2, space="PSUM") as psum,
            tc.tile_pool(name="opool", bufs=1) as opool,
        ):
            x1t = [
                xpool.tile([P, CFD], bf16, tag=f"x1t{s}", name=f"x1t{s}")
                for s in range(SUB)
            ]
            x2t = [
                xpool.tile([P, WB * BFD], bf16, tag=f"x2t{s}", name=f"x2t{s}")
                for s in range(SUB)
            ]
            # contiguous per-tile transfers, spread across engine queues so
            # the pieces stream in parallel and complete in consumption order
            # gpsimd + scalar DMA queues each sustain ~150GB/s concurrently
            # (sync's is much slower) - alternate so each chunk's two pieces
            # land at matching FIFO positions and arrive together, in order.
            for s in range(SUB):
                a, b = (nc.gpsimd, nc.scalar) if s % 2 == 0 else (nc.scalar, nc.gpsimd)
                a.dma_start(x2t[s][:, :], x2_dram[s][:, :])
                b.dma_start(x1t[s][:, :], x1_dram[s][:, :])

            c1_ps = psum.tile([P, P], f32, tag="c1")
            c2_ps = psum.tile([P, P], f32, tag="c2")
            n1 = (NSLOT - 1) * SUB * PMT
            n2 = SUB * PMT
            k1 = k2 = 0
            for s in range(SUB):
                for j in range(NSLOT):
                    idx = s * NSLOT + j
                    diff = dpool.tile([P, CFD], bf16, tag="diff", name=f"diff{idx}")
                    sign = spool.tile([P, CFD], bf16, tag="sign", name=f"sign{idx}")
                    nc.vector.tensor_tensor(
                        diff[:, :],
                        x2t[s][:, j * BFD:j * BFD + CFD],
                        x1t[s][:, :],
                        op=mybir.AluOpType.subtract,
                    )
                    if idx % 3 == 1:
                        # DVE route: clamp nonzero integer diffs to +-1
                        nc.vector.tensor_scalar(
                            sign[:, :],
                            diff[:, :],
                            1.0,
                            -1.0,
                            op0=mybir.AluOpType.min,
                            op1=mybir.AluOpType.max,
                        )
                    else:
                        nc.scalar.activation(
                            sign[:, :],
                            diff[:, :],
                            mybir.ActivationFunctionType.Sign,
                        )
                    for m in range(PMT):
                        w_tile = sign[:, m * P:(m + 1) * P]
                        if j < NSLOT - 1:
                            st, sp = k1 == 0, k1 == n1 - 1
                            k1 += 1
                            acc = c1_ps
                        else:
                            st, sp = k2 == 0, k2 == n2 - 1
                            k2 += 1
                            acc = c2_ps
                        nc.tensor.matmul(
                            acc[:, :], w_tile, w_tile, start=st, stop=sp
                        )

            out_sb = opool.tile([P, 2 * P], f32)
            nc.vector.tensor_copy(out_sb[:, 0:P], c1_ps[:, :])
            nc.scalar.activation(
                out_sb[:, P:2 * P],
                c2_ps[:, :],
                mybir.ActivationFunctionType.Copy,
            )
            nc.sync.dma_start(out_dram[:, :], out_sb[:, :])

    nc.compile()
    return nc


def _build_nc_raw():
    """Raw bacc pipeline (no TileContext): manual semaphores, so the ~10us
    Tile epilogue (drain + 2 all-engine barriers + sem clears) disappears.

    Engine plan per core (16 chunks, idx = s*4+j):
      gpsimd : 4 input DMAs (its HW queue, ~160GB/s)
      scalar : 4 input DMAs + 11 ACT Sign chunks + final c2 copy
      vector : 16 subtracts + 5 clamp chunks + final c1 copy
      tensor : 256 paired Gram matmuls
      sync   : output DMA
    """
    nc = bacc.Bacc(
        "TRN2",
        target_bir_lowering=False,
        debug=False,
        num_devices=NCORES,
    )
    f32 = mybir.dt.float32
    bf16 = mybir.dt.bfloat16
    x1_dram = [
        nc.dram_tensor(f"x1s{s}", [P, CFD], bf16, kind="ExternalInput")
        for s in range(SUB)
    ]
    x2_dram = [
        nc.dram_tensor(f"x2s{s}", [P, WB * BFD], bf16, kind="ExternalInput")
        for s in range(SUB)
    ]
    out_dram = nc.dram_tensor("out", [P, 2 * P], f32, kind="ExternalOutput")

    NB = 4                       # diff/sign ring depth
    NCHUNK = SUB * NSLOT
    is_dve = [idx % 3 == 1 for idx in range(NCHUNK)]
    # running counts of ACT-/DVE-route chunks with index <= idx
    a_cnt, d_cnt, a, d = [], [], 0, 0
    for idx in range(NCHUNK):
        if is_dve[idx]:
            d += 1
        else:
            a += 1
        a_cnt.append(a)
        d_cnt.append(d)

    x1t = [nc.alloc_sbuf_tensor(f"x1t{s}", [P, CFD], bf16) for s in range(SUB)]
    x2t = [nc.alloc_sbuf_tensor(f"x2t{s}", [P, WB * BFD], bf16) for s in range(SUB)]
    diff = [nc.alloc_sbuf_tensor(f"diff{k}", [P, CFD], bf16) for k in range(NB)]
    sign = [nc.alloc_sbuf_tensor(f"sign{k}", [P, CFD], bf16) for k in range(NB)]
    out_sb = nc.alloc_sbuf_tensor("out_sb", [P, 2 * P], f32)
    c1_ps = nc.alloc_psum_tensor("c1_ps", [P, P], f32)
    c2_ps = nc.alloc_psum_tensor("c2_ps", [P, P], f32)

    n1 = (NSLOT - 1) * SUB * PMT
    n2 = SUB * PMT

    from contextlib import ExitStack

    with ExitStack() as ctx:
        gd = [ctx.enter_context(nc.semaphore(f"gd{s}")) for s in range(SUB)]
        sd = [ctx.enter_context(nc.semaphore(f"sd{s}")) for s in range(SUB)]
        diff_sem = ctx.enter_context(nc.semaphore("diff_sem"))
        act_sem = ctx.enter_context(nc.semaphore("act_sem"))
        dvs_sem = ctx.enter_context(nc.semaphore("dvs_sem"))
        mm_sem = ctx.enter_context(nc.semaphore("mm_sem"))
        cp_sem = ctx.enter_context(nc.semaphore("cp_sem"))
        odma_sem = ctx.enter_context(nc.semaphore("odma_sem"))
        block = ctx.enter_context(nc.Block())

        @block.gpsimd
        def _(gpsimd):
            for s in range(SUB):
                src = x2_dram[s] if s % 2 == 0 else x1_dram[s]
                dst = x2t[s] if s % 2 == 0 else x1t[s]
                gpsimd.dma_start(dst.ap(), src.ap()).then_inc(gd[s], 16)

        @block.scalar
        def _(scalar):
            for s in range(SUB):
                src = x1_dram[s] if s % 2 == 0 else x2_dram[s]
                dst = x1t[s] if s % 2 == 0 else x2t[s]
                scalar.dma_start(dst.ap(), src.ap()).then_inc(sd[s], 16)
            for idx in range(NCHUNK):
                if is_dve[idx]:
                    continue
                scalar.wait_ge(diff_sem, idx + 1)
                if idx >= NB:
                    scalar.wait_ge(mm_sem, idx - NB + 1)
                scalar.activation(
                    sign[idx % NB].ap(),
                    diff[idx % NB].ap(),
                    mybir.ActivationFunctionType.Sign,
                ).then_inc(act_sem, 1)
            scalar.wait_ge(mm_sem, NCHUNK)
            scalar.activation(
                out_sb.ap()[:, P:2 * P],
                c2_ps.ap(),
                mybir.ActivationFunctionType.Copy,
            ).then_inc(cp_sem, 1)

        @block.vector
        def _(vector):
            for idx in range(NCHUNK):
                s, j = divmod(idx, NSLOT)
                if j == 0:
                    vector.wait_ge(gd[s], 16)
                    vector.wait_ge(sd[s], 16)
                prev = idx - NB
                if prev >= 0 and not is_dve[prev]:
                    vector.wait_ge(act_sem, a_cnt[prev])
                vector.tensor_tensor(
                    diff[idx % NB].ap(),
                    x2t[s].ap()[:, j * BFD:j * BFD + CFD],
                    x1t[s].ap(),
                    op=mybir.AluOpType.subtract,
                ).then_inc(diff_sem, 1)
                if is_dve[idx]:
                    vector.wait_ge(diff_sem, idx + 1)
                    if idx >= NB:
                        vector.wait_ge(mm_sem, idx - NB + 1)
                    vector.tensor_scalar(
                        sign[idx % NB].ap(),
                        diff[idx % NB].ap(),
                        1.0,
                        -1.0,
                        op0=mybir.AluOpType.min,
                        op1=mybir.AluOpType.max,
                    ).then_inc(dvs_sem, 1)
            vector.wait_ge(mm_sem, NCHUNK)
            vector.tensor_copy(out_sb.ap()[:, 0:P], c1_ps.ap()).then_inc(cp_sem, 1)

        @block.tensor
        def _(tensor):
            k1 = k2 = 0
            for idx in range(NCHUNK):
                j = idx % NSLOT
                tensor.wait_ge(act_sem, a_cnt[idx])
                tensor.wait_ge(dvs_sem, d_cnt[idx])
                for m in range(PMT):
                    w_tile = sign[idx % NB].ap()[:, m * P:(m + 1) * P]
                    if j < NSLOT - 1:
                        st, sp = k1 == 0, k1 == n1 - 1
                        k1 += 1
                        acc = c1_ps
                    else:
                        st, sp = k2 == 0, k2 == n2 - 1
                        k2 += 1
                        acc = c2_ps
                    mm = tensor.matmul(
                        acc.ap(), w_tile, w_tile, start=st, stop=sp
                    )
                    if m == PMT - 1:
                        mm.then_inc(mm_sem, 1)

        @block.sync
        def _(sync):
            sync.wait_ge(cp_sem, 2)
            sync.dma_start(out_dram.ap(), out_sb.ap()).then_inc(odma_sem, 16)
            sync.wait_ge(odma_sem, 16)

    nc.compile()
    return nc


def _get_nc():
    # The TileContext build measures ~43us vs ~46us for the manual-semaphore
    # build (_build_nc_raw): the end-of-kernel barrier/sem traffic is
    # Bass/NRT-level (present in both), and Tile's finer-grained deps
    # overlap the chunk pipeline better.
    if "nc" not in _CACHE:
        _CACHE["nc"] = _build_nc()
    return _CACHE["nc"]


def _ranks(X):
    """Per-(t,i)-column batch ranks, 0..B-1, exact in bf16."""
    order = np.argsort(X, axis=0, kind="stable")
    ranks = np.empty_like(order)
    np.put_along_axis(
        ranks, order, np.arange(B, dtype=order.dtype)[:, None, None], axis=0
    )
    return ranks.astype(np.float32)


def _to_sbuf_layout(blocks):
    nb = blocks.shape[0]
    return np.ascontiguousarray(
        blocks.reshape(nb, TH, P, N)
        .transpose(2, 0, 1, 3)
        .reshape(P, nb * BFD)
        .astype(ml_dtypes.bfloat16)
    )


def _prep_core_inputs(R, c):
    r = np.roll(R, -(NSLOT * c + 1), axis=0)
    ext = np.concatenate([r, r[: NSLOT - 1]], axis=0)  # 67 blocks
    ins = {}
    for s in range(SUB):
        ins[f"x1s{s}"] = _to_sbuf_layout(R[CB * s:CB * (s + 1)])
        ins[f"x2s{s}"] = _to_sbuf_layout(ext[CB * s:CB * s + WB])
    return ins


def _tie_correction(X, ranks):
    """Exact fix for within-column value ties: the rank-sign kernel counts
    sign(rank diff)=+-1 where the true sign is 0."""
    C_fix = np.zeros((N, N), dtype=np.float64)
    Xs = np.sort(X, axis=0)
    t_idx, i_idx = np.nonzero((Xs[1:] == Xs[:-1]).any(axis=0))
    events = {}
    for t, i in zip(t_idx, i_idx):
        col = X[:, t, i]
        order = np.argsort(col, kind="stable")
        sc = col[order]
        for k in np.nonzero(sc[1:] == sc[:-1])[0]:
            a, b = order[k], order[k + 1]
            events.setdefault((min(a, b), max(a, b), t), []).append(i)
    for (a, b, t), cols in events.items():
        shat = np.sign(ranks[b, t, :] - ranks[a, t, :])
        W = np.outer(shat, shat)
        mask = np.zeros((N, N), dtype=bool)
        mask[cols, :] = True
        mask[:, cols] = True
        C_fix += W * mask
    return C_fix.astype(np.float32)


def kernel(**inputs) -> np.ndarray:
    X = np.asarray(inputs["inputs"], dtype=np.float32)
    R = _ranks(X)
    nc = _get_nc()
    in_maps = [_prep_core_inputs(R, c) for c in range(NCORES)]
    res = run_bass_kernel_spmd(nc, in_maps, core_ids=list(range(NCORES)))
    C_half = np.zeros((N, N), dtype=np.float32)
    for c, r in enumerate(res.results):
        o = r["out"]
        C_half += o[0:N, 0:N] + o[N:P, N:P]
        w = np.float32(0.5) if c == NCORES - 1 else np.float32(1.0)
        C_half += (o[0:N, P:P + N] + o[N:P, P + N:2 * P]) * w
    C_half -= _tie_correction(X, R)
    D = (1.0 - C_half / np.float32(PAIRS_HALF)) * (
        1.0 - np.eye(N, dtype=np.float32)
    )
    return np.ascontiguousarray(
        np.broadcast_to(D[None].astype(np.float32), (B, N, N))
    )
